# revision 7
# baseline (speedup 1.0000x reference)
"""Trainium2 Bass kernel for nn_DeterministicAdjacency (gnn_message_passing).

Math (reference):
    hi = z @ W1[:D]; hj = z @ W1[D:]                      # (K, E)
    logits[i,j] = sum_e W2[e] * silu(hi[i,e] + hj[j,e] + b1[e])
    out = softmax(logits, axis=-1)

Factorized algorithm (this kernel):
    silu(x) = x/2 + g(x) with g even; fit g(x) ~ g0 + sum_f gam_f*cos(om_f*x)
    (F=4 cosines, max abs err ~2e-3 on |x|<=9).  With a = hi + b1, b = hj:
      cos(om(a+b)) = cos(om a)cos(om b) - sin(om a)sin(om b)
    => logits[i,j] ~ rowconst_i + vlin_j
                     + sum_{e,f} gam_f W2[e] [cos_a cos_b - sin_a sin_b]
    rowconst_i (a-side linear + g0 terms) drops under row-softmax;
    vlin_j = sum_e W2[e] b_je / 2 = (z @ (W1b@W2)/2)_j stays (rank-1).

    => logits = U' @ V^T + vlin: a matmul with contraction 2*F*E = 512.
    Trig is evaluated on K*E points instead of K^2*E silu points.

Per-core structure (rows sharded 8 ways, 256 rows = 2 i-tiles per core):
  * c-chunk f (128 partitions) = [(cos, e<64); (sin, e<64)] at freq om_f.
  * V_f [128, 2048] fp16 moving operand: PE proj with dup-column stationary
    [W1b|W1b] (om_f-prescaled for f>=1), then ACT Sin.  HW Sin is only
    valid on [-pi,pi]; args reach 8.9 rad, so f>=1 chunks go through the
    custom DVE ADD_RANGE_WRAP (1 instr: y = x + s0(phase), +-2pi wrap).
    f=0 fits in range and takes scale/bias inside the ACT instruction.
  * U'_f [128, 256] fp16 stationary: same pipeline on zcT, then a
    per-partition scale by +-gam_f*W2[e] (DVE).
  * acc[u][half] [128, 2, 512] PSUM: rank-1 vlin init matmul (stationary
    outer((W1b@W2)/2, ones) against zT), + 4 chunk matmuls.
  * j is processed in 2 halves so PSUM fits acc0+acc1+proj simultaneously:
    per half: proj -> sin -> chunk MMs -> per-half Exp with accum_out
    (partial row sums; logits are O(+-4) so max-subtraction is skipped).
    Halves' sums combine, DVE reciprocal, then normalize (DVE + gpsimd in
    parallel) and store, spread over the qSync/qScalar/qPool DMA queues.
  * inputs are batched into 3 side tensors (zT quarters on qSync, Wcat on
    qScalar, zcT+Bcat on qPool) so the first projection starts ~1.5us in.
"""

import math

import numpy as np

import concourse.bass as bass
import concourse.bacc as bacc
import concourse.mybir as mybir
from concourse import tile
from concourse.bass_utils import run_bass_kernel_spmd
from concourse.dve_ops import ADD_RANGE_WRAP

K, D, E = 2048, 128, 64
NCORES = 8
R = K // NCORES            # 256 rows per core
NF = 4                     # cosine terms
F32 = mybir.dt.float32
F16 = mybir.dt.float16
AF = mybir.ActivationFunctionType
ALU = mybir.AluOpType

# fit of g(x) = silu(x) - x/2 on [-9, 9], gaussian-weighted LSQ (F=4).
OM = np.array([0.31411689, 0.89155844, 1.49059269, 2.17966537])
GAM = np.array([-2.16396998, -0.22420055, -0.03599722, -0.00465312])
# Bcat columns: 0..3 U-side sin bias (om_f*b1 + phase), 4..7 V-side
# (phase only), 8..11 U' scale (+-gam_f*W2[e]).
BU, BV, SW = 0, NF, 2 * NF


def build_nc() -> bass.Bass:
    nc = bacc.Bacc(None, target_bir_lowering=False)
    zT_d = nc.declare_dram_parameter("zT", [D, K], F16, isOutput=False)
    zcT_d = nc.declare_dram_parameter("zcT", [D, R], F16, isOutput=False)
    # Wcat: 0..3 V-proj stationaries (g0 unscaled, g>=1 om-scaled dup
    # [W1b|W1b]), 4..7 U-proj ([W1a|W1a]), 8 vlin rank-1 (outer(wv, ones)).
    wcat_d = nc.declare_dram_parameter("wcat", [D, 9, 128], F16, isOutput=False)
    bcat_d = nc.declare_dram_parameter("bcat", [128, 3 * NF], F32, isOutput=False)
    out_d = nc.declare_dram_parameter("out", [R, K], F32, isOutput=True)

    with tile.TileContext(nc) as tc:
        with (
            tc.tile_pool(name="singles", bufs=1) as singles,
            tc.tile_pool(name="scratch", bufs=2) as scr,
        ):
            zT = singles.tile([128, K], F16)
            zcT = singles.tile([128, R], F16)
            wcat = singles.tile([128, 9, 128], F16)
            bcat = singles.tile([128, 3 * NF], F32)
            V = [singles.tile([128, K], F16, name=f"V{f}") for f in range(NF)]
            up = singles.tile([128, NF, R], F16)
            ex = [singles.tile([128, K], F32, name=f"ex{u}") for u in range(2)]

            # input DMAs: zcT + zT quarters on qSync; wcat/bcat on qScalar
            nc.sync.dma_start(out=zcT[:], in_=zcT_d[:])
            nc.scalar.dma_start(out=wcat[:], in_=wcat_d[:])
            nc.scalar.dma_start(out=bcat[:], in_=bcat_d[:])
            for q in range(4):
                sl = slice(q * 512, (q + 1) * 512)
                nc.sync.dma_start(out=zT[:, sl], in_=zT_d[:, sl])

            # ---- U side: 4 stationary chunks U'_f [128, 256] fp16 ----
            with tc.tile_pool(name="pu", bufs=1, space="PSUM") as pu:
                xu = pu.tile([128, NF, R], F32)
                for g in range(NF):
                    nc.tensor.matmul(
                        xu[:, g, :], wcat[:, 4 + g, :], zcT[:],
                        start=True, stop=True,
                    )
                for f in range(NF):
                    usin = scr.tile([128, R], F32, tag="usin")
                    if f == 0:
                        nc.scalar.activation(
                            out=usin[:], in_=xu[:, 0, :], func=AF.Sin,
                            scale=float(OM[0]), bias=bcat[:, BU : BU + 1],
                        )
                    else:
                        uwr = scr.tile([128, R], F32, tag="uwr")
                        nc.vector._custom_dve(
                            ADD_RANGE_WRAP, out=uwr[:], in0=xu[:, f, :],
                            s0=bcat[:, BU + f : BU + f + 1], s1=math.pi,
                            imm2=2 * math.pi,
                        )
                        nc.scalar.activation(out=usin[:], in_=uwr[:], func=AF.Sin)
                    nc.vector.tensor_scalar_mul(
                        out=up[:, f, :], in0=usin[:],
                        scalar1=bcat[:, SW + f : SW + f + 1],
                    )

            # ---- V side + chunk matmuls, pipelined over two j-halves ----
            # PE emission is software-pipelined: chunk-f matmuls are placed
            # right after the f+1 projection so the in-order PE queue stays
            # dense (p-state ramps to 2.4GHz only after ~3us continuous).
            # h0 accumulators are copied to SBUF (DVE) so their PSUM banks
            # recycle for h1 while ALL exps run at the end (2 ACT table
            # loads total instead of 5).
            tots = {}
            sx = [singles.tile([128, 1024], F32, name=f"sx{u}") for u in range(2)]
            with (
                tc.tile_pool(name="pacc", bufs=1, space="PSUM") as pacc,
                tc.tile_pool(name="pxv", bufs=1, space="PSUM") as pxv,
            ):
                accs = {}

                def proj(g, h):
                    xv = pxv.tile([128, 2, 512], F32, tag="xv", bufs=2)
                    for tt in range(2):
                        sl = slice(h * 1024 + tt * 512, h * 1024 + (tt + 1) * 512)
                        nc.tensor.matmul(
                            xv[:, tt, :], wcat[:, g, :], zT[:, sl],
                            start=True, stop=True,
                        )
                    H = h * 1024
                    if g == 0:
                        nc.scalar.activation(
                            out=V[0][:, H : H + 1024].rearrange(
                                "p (t j) -> p t j", t=2
                            ),
                            in_=xv[:], func=AF.Sin,
                            scale=float(OM[0]), bias=bcat[:, BV : BV + 1],
                        )
                    else:
                        vwr = scr.tile([128, 1024], F32, tag="vwr")
                        nc.vector._custom_dve(
                            ADD_RANGE_WRAP,
                            out=vwr.rearrange("p (t j) -> p t j", t=2),
                            in0=xv[:], s0=bcat[:, BV + g : BV + g + 1],
                            s1=math.pi, imm2=2 * math.pi,
                        )
                        nc.scalar.activation(
                            out=V[g][:, H : H + 1024], in_=vwr[:], func=AF.Sin,
                        )

                def inits(h):
                    for u in range(2):
                        acc = pacc.tile(
                            [128, 2, 512], F32, tag=f"acc{u}", bufs=1,
                            name=f"acc{u}h{h}",
                        )
                        accs[(u, h)] = acc
                        for tt in range(2):
                            sl = slice(
                                h * 1024 + tt * 512, h * 1024 + (tt + 1) * 512
                            )
                            nc.tensor.matmul(
                                acc[:, tt, :], wcat[:, 8, :], zT[:, sl],
                                start=True, stop=False,
                            )

                def chunk_mm(f, h):
                    for u in range(2):
                        for tt in range(2):
                            sl = slice(
                                h * 1024 + tt * 512, h * 1024 + (tt + 1) * 512
                            )
                            nc.tensor.matmul(
                                accs[(u, h)][:, tt, :],
                                up[:, f, u * 128 : (u + 1) * 128],
                                V[f][:, sl],
                                start=False, stop=(f == NF - 1),
                            )

                # h1 projections are pulled forward between h0 chunk-MMs
                # so the last sins (critical path into the exps) land early.
                proj(0, 0)
                proj(1, 0)
                inits(0)
                chunk_mm(0, 0)
                proj(2, 0)
                chunk_mm(1, 0)
                proj(3, 0)
                proj(0, 1)
                chunk_mm(2, 0)
                proj(1, 1)
                chunk_mm(3, 0)
                # free h0 PSUM: stage into SBUF for the end-exps
                for u in range(2):
                    nc.vector.tensor_copy(
                        out=sx[u].rearrange("p (t j) -> p t j", t=2),
                        in_=accs[(u, 0)][:],
                    )
                inits(1)
                chunk_mm(0, 1)
                proj(2, 1)
                chunk_mm(1, 1)
                proj(3, 1)
                chunk_mm(2, 1)
                chunk_mm(3, 1)

                # ---- all exps (one table switch), combine, normalize ----
                # gate = exact 1.0, data-dependent on the LAST sin: passed as
                # the exps' scale AP so the tile scheduler cannot hoist an
                # Exp between Sins (each hoist would cost 2x1.28us of ACT
                # table reloads).
                gate = scr.tile([128, 1], F32, tag="gate", bufs=1)
                nc.vector.tensor_scalar(
                    out=gate[:], in0=V[NF - 1][:, K - 1 : K],
                    scalar1=0.0, scalar2=1.0, op0=ALU.mult, op1=ALU.add,
                )
                for u in range(2):
                    for h in range(2):
                        tot = scr.tile([128, 1], F32, tag=f"tot{u}{h}", bufs=1)
                        tots[(u, h)] = tot
                        src_ap = (
                            sx[u].rearrange("p (t j) -> p t j", t=2)
                            if h == 0 else accs[(u, 1)][:]
                        )
                        nc.scalar.activation(
                            out=ex[u][:, h * 1024 : (h + 1) * 1024].rearrange(
                                "p (t j) -> p t j", t=2
                            ),
                            in_=src_ap, func=AF.Exp,
                            scale=gate[:],
                            accum_out=tot[:],
                        )
                qrot = (nc.sync, nc.scalar, nc.gpsimd)
                qi = 0
                for u in range(2):
                    rec = scr.tile([128, 1], F32, tag=f"rec{u}", bufs=1)
                    nc.vector.tensor_scalar_add(
                        out=rec[:], in0=tots[(u, 0)][:],
                        scalar1=tots[(u, 1)][:],
                    )
                    nc.vector.reciprocal(out=rec[:], in_=rec[:])
                    for c in range(4):
                        sl = slice(c * 512, (c + 1) * 512)
                        nc.vector.tensor_scalar_mul(
                            out=ex[u][:, sl], in0=ex[u][:, sl], scalar1=rec[:]
                        )
                        qrot[qi % 3].dma_start(
                            out=out_d[u * 128 : (u + 1) * 128, sl],
                            in_=ex[u][:, sl],
                        )
                        qi += 1
    nc.finalize()
    return nc


_CACHE: dict = {}


def _get_nc() -> bass.Bass:
    if "nc" not in _CACHE:
        _CACHE["nc"] = build_nc()
    return _CACHE["nc"]


def make_in_maps(z, W1, b1, W2):
    z = np.asarray(z, np.float32)
    W1 = np.asarray(W1, np.float32)
    b1 = np.asarray(b1, np.float32)
    w2 = np.asarray(W2, np.float32).reshape(-1)

    W1a, W1b = W1[:D], W1[D:]
    dup = lambda M: np.concatenate([M, M], axis=1)  # (D, 128)
    phase = np.concatenate(
        [np.full(E, np.pi / 2, np.float32), np.zeros(E, np.float32)]
    )
    b1dup = np.tile(b1, 2)

    scales = [1.0, OM[1], OM[2], OM[3]]   # g0 scaled inside ACT
    wcat = np.stack(
        [s * dup(W1b) for s in scales]
        + [s * dup(W1a) for s in scales]
        + [np.tile(((W1b @ w2) / 2.0)[:, None], (1, 128))],
        axis=1,
    ).astype(np.float16)                  # (D, 9, 128)

    bcat = np.zeros((128, 3 * NF), np.float32)
    for f in range(NF):
        bcat[:, BU + f] = OM[f] * b1dup + phase
        bcat[:, BV + f] = phase
        bcat[:, SW + f] = np.concatenate([GAM[f] * w2, -GAM[f] * w2])

    zT16 = np.ascontiguousarray(z.astype(np.float16).T)  # (D, K)

    in_maps = []
    for c in range(NCORES):
        in_maps.append(
            {
                "zT": zT16,
                "zcT": np.ascontiguousarray(zT16[:, c * R : (c + 1) * R]),
                "wcat": np.ascontiguousarray(wcat),
                "bcat": np.ascontiguousarray(bcat),
            }
        )
    return in_maps


def run(inputs: dict, trace: bool = False):
    """Run the bass kernel; returns (full_output, BassKernelResults)."""
    nc = _get_nc()
    in_maps = make_in_maps(inputs["z"], inputs["W1"], inputs["b1"], inputs["W2"])
    res = run_bass_kernel_spmd(nc, in_maps, list(range(NCORES)), trace=trace)
    full = np.concatenate([res.results[c]["out"] for c in range(NCORES)], axis=0)
    return full, res


def kernel(**inputs) -> np.ndarray:
    full, _ = run(inputs, trace=False)
    return full
